# revision 14
# baseline (speedup 1.0000x reference)
"""Trainium2 Bass kernel for nn_InvDiff: d = diff(x, axis=1), y = restore(d).

Math: the reference computes
    d[b, i, f] = x[b, i+1, f] - x[b, i, f]              (i in [0, L-2])
    y[b, i, f] = cumsum(d[:, :-1])[b, i, f]             (i in [0, L-3])
    y[b, L-2, f] = 0
The cumsum telescopes: cumsum(d)[b, i, f] = x[b, i+1, f] - x[b, 0, f].
So both outputs are pure shifted elementwise subtractions -> memory bound.

Distribution: batch axis (64) sharded 8 ways across 8 NeuronCores; each core
handles 8 batches independently (pure data parallelism, no communication).

Per-core layout: each batch's (L, F) block is viewed flat (1,048,576 f32) and
split into 128 partitions x 8192 contiguous elements.  The lag-256 shifted
operand is made partition-local by loading each partition row with a
256-element overlap into the next row's span ([[8192,128],[1,8448]] AP), so
d and y are each ONE big DVE tensor_sub per chunk.  y's subtrahend
(x[b,0,:], periodic along the flat axis with period 256) is a host-provided
[128, 256] tile read through a stride-0 broadcast AP.
"""

import numpy as np

import concourse.bacc as bacc
import concourse.bass as bass
import concourse.mybir as mybir
import concourse.tile as tile
from concourse.ap import AP
from concourse.bass_utils import run_bass_kernel_spmd

# Problem shape (hardcoded per contract).
B, L, F = 64, 4096, 256
N_CORES = 8
NB = B // N_CORES          # batches per core = 8
P = 128                    # SBUF partitions
LF = L * F                 # 1_048_576 elems per batch
SPAN = LF // P             # 8192 elems per partition row
OV = F                     # 256-elem overlap (the diff lag)
OUT_LF = (L - 1) * F       # 1_048_320 elems per output batch
CC = 4096                  # free-dim chunk of the compute/stores
NCH = SPAN // CC           # chunks per batch (2 at CC=4096)
REPS = CC // F             # repeats of the x0 row per chunk
GRP = 32                   # partition rows per store dma_start (4 groups)
FP32 = mybir.dt.float32

_CACHE = {}


def _build():
    nc = bacc.Bacc(
        "TRN2",
        target_bir_lowering=False,
        debug=False,
        num_devices=N_CORES,
    )
    # Flat with OV elements of tail padding so every batch (including the
    # last) can use the uniform [P, SPAN+OV] overlapped load.  A ragged
    # 127-row load is NOT sprayed across SDMA engines (one engine, ~27
    # GB/s) and was costing ~200us of tail stall.
    x_h = nc.dram_tensor("x", (NB * LF + OV,), FP32, kind="ExternalInput")
    # Host lays x0 out as [P, NB*F]: one contiguous SBUF-shaped block so all
    # batches' pivot rows load in a single DMA up front.
    x0_h = nc.dram_tensor("x0", (P, NB * F), FP32, kind="ExternalInput")
    d_h = nc.dram_tensor("d", (NB, L - 1, F), FP32, kind="ExternalOutput")
    y_h = nc.dram_tensor("y", (NB, L - 1, F), FP32, kind="ExternalOutput")

    with tile.TileContext(nc) as tc:
        with (
            tc.tile_pool(name="xt", bufs=2) as xpool,
            tc.tile_pool(name="dt", bufs=4) as dpool,
            tc.tile_pool(name="yt", bufs=4) as ypool,
            tc.tile_pool(name="x0t", bufs=1) as x0pool,
        ):
            x0t = x0pool.tile([P, NB * F], FP32)
            nc.scalar.dma_start(x0t[:, :], AP(x0_h, 0, [[NB * F, P], [1, NB * F]]))

            for b in range(NB):
                xb = b * LF
                t = xpool.tile([P, SPAN + OV], FP32)
                # Overlapping rows: partition p holds flat[p*SPAN : p*SPAN+SPAN+OV].
                # Row 127's overlap reads the next batch's head (or the tail
                # padding for the last batch); those values are never stored.
                # Column-split into NCH pieces: chunk j's subs depend only
                # on the columns loaded so far, so the first stores start
                # ~half a load earlier and load packets interleave with
                # store packets at finer granularity.
                for lj in range(NCH):
                    # Piece 0 carries the OV overlap so chunk 0's subs
                    # depend on piece 0 alone.
                    lc0 = 0 if lj == 0 else lj * CC + OV
                    lw = (CC + OV) if lj == 0 else CC
                    nc.sync.dma_start(
                        t[:, lc0 : lc0 + lw],
                        AP(x_h, xb + lc0, [[SPAN, P], [1, lw]]),
                    )

                ob = b * OUT_LF
                for j in range(NCH):
                    c0 = j * CC
                    dt_ = dpool.tile([P, CC], FP32)
                    yt = ypool.tile([P, CC], FP32)
                    nc.vector.tensor_sub(
                        dt_[:, :], t[:, c0 + OV : c0 + OV + CC], t[:, c0 : c0 + CC]
                    )
                    nc.vector.tensor_sub(
                        yt[:, :].rearrange("p (r f) -> p r f", f=F),
                        t[:, c0 + OV : c0 + OV + CC].rearrange(
                            "p (r f) -> p r f", f=F
                        ),
                        x0t[:, b * F : (b + 1) * F]
                        .unsqueeze(1)
                        .to_broadcast([P, REPS, F]),
                    )
                    # Row 127 is ragged: d output ends at 127*SPAN + 7936,
                    # y valid data ends 256 earlier (y[b, L-2, :] = 0 comes
                    # from the pre-zeroed output buffer; both run paths
                    # zero-fill ExternalOutput buffers before execution).
                    # Chunks fully inside the valid region include row 127
                    # in the last group; the ragged remainders go on the
                    # (otherwise idle) scalar HWDGE ring.
                    full127d = c0 + CC <= SPAN - OV
                    full127y = c0 + CC <= SPAN - OV - F
                    # All bulk stores go through SWDGE (gpsimd).  Each SWDGE
                    # dma_start is serviced by ONE SDMA engine (~27 GB/s);
                    # successive dma_starts round-robin across the 16
                    # engines, so split each store into GRP-row groups to
                    # keep all 16 engines fed.
                    pd = P if full127d else P - 1
                    py = P if full127y else P - 1
                    for r0 in range(0, pd, GRP):
                        r1 = min(r0 + GRP, pd)
                        nc.gpsimd.dma_start(
                            AP(d_h, ob + c0 + r0 * SPAN, [[SPAN, r1 - r0], [1, CC]]),
                            dt_[r0:r1, :],
                            single_packet=True,
                        )
                    for r0 in range(0, py, GRP):
                        r1 = min(r0 + GRP, py)
                        nc.gpsimd.dma_start(
                            AP(y_h, ob + c0 + r0 * SPAN, [[SPAN, r1 - r0], [1, CC]]),
                            yt[r0:r1, :],
                            single_packet=True,
                        )
                    if not full127d:
                        w = SPAN - OV - c0
                        nc.scalar.dma_start(
                            AP(d_h, ob + (P - 1) * SPAN + c0, [[SPAN, 1], [1, w]]),
                            dt_[P - 1 : P, 0:w],
                        )
                    if not full127y:
                        w = SPAN - OV - F - c0
                        nc.scalar.dma_start(
                            AP(y_h, ob + (P - 1) * SPAN + c0, [[SPAN, 1], [1, w]]),
                            yt[P - 1 : P, 0:w],
                        )

    nc.compile()
    return nc


def get_nc():
    if "nc" not in _CACHE:
        _CACHE["nc"] = _build()
    return _CACHE["nc"]


def _in_maps(x: np.ndarray):
    x = np.ascontiguousarray(x, dtype=np.float32)
    maps = []
    for i in range(N_CORES):
        xs = x[i * NB : (i + 1) * NB]
        xf = np.zeros(NB * LF + OV, dtype=np.float32)
        xf[: NB * LF] = xs.ravel()
        # [P, NB*F]: row p holds all batches' pivot rows x[b, 0, :].
        x0 = np.broadcast_to(
            xs[:, 0, :].reshape(1, NB * F), (P, NB * F)
        ).copy()
        maps.append({"x": xf, "x0": x0})
    return maps


def run(x: np.ndarray, trace: bool = False):
    nc = get_nc()
    res = run_bass_kernel_spmd(
        nc, _in_maps(x), core_ids=list(range(N_CORES)), trace=trace
    )
    d = np.concatenate([r["d"] for r in res.results], axis=0)
    y = np.concatenate([r["y"] for r in res.results], axis=0)
    return (d, y), res


def kernel(x: np.ndarray):
    (d, y), _ = run(x, trace=False)
    return d, y

